# revision 5
# baseline (speedup 1.0000x reference)
"""ConVIRT loss (NT-Xent both directions) on 8 Trainium2 NeuronCores.

Strategy: shard img rows across 8 cores; each core computes its
[j=8192(text), i=1024(img-block)] slice of the similarity matrix as
simT = textT.T @ imgT with fp8 DoubleRow matmuls (K packed 2x256), then
e = exp(sim/(QS^2*TEMP)) on the ACT engine.  Both reductions of e come
cheap:
  - colsum over i (free axis) rides the ACT `accum_out` per j-tile
  - rowsum over j (partitions) is a DoubleRow ones-matmul over e packed
    as [128, 2(jt parity), 1024] fp8, PSUM-accumulated across all tiles
The host pre-normalizes both tensors (fp32), quantizes to fp8e4 scaled
by QS=8, and ships the operands already transposed+DoubleRow-packed, so
the device spends no cycles on norms, casts, or transposes.  The diag
term and the final log/mean run on host (O(N*D) / O(N)).
"""

import numpy as np
import ml_dtypes

import concourse.bacc as bacc
import concourse.tile as tile
import concourse.mybir as mybir
from concourse.bass_utils import run_bass_kernel_spmd

N, D = 8192, 512
CORES = 8
BLK = N // CORES          # 1024 img rows per core
NT = N // 128             # 64 text (j) tiles
NB = NT // 2              # 32 jt pairs
IC = BLK // 512           # 2 moving-free chunks of 512
KP = 2                    # kpairs: D=512 -> 2 x (2x128) DoubleRow groups
NCH = 8                   # text DMA chunks per kpair (8 jt each)
CHJ = NT // NCH * 128     # 1024 j columns per chunk
TEMP, ALPHA, EPS = 0.1, 0.75, 1e-8
QS = 8.0                  # fp8 quant scale: q = fp8(QS * normalized)
ACT_SCALE = 1.0 / (QS * QS * TEMP)

f32 = mybir.dt.float32
fp8 = mybir.dt.float8e4
AF = mybir.ActivationFunctionType
DR = mybir.MatmulPerfMode.DoubleRow
np8 = ml_dtypes.float8_e4m3

_CACHE = {}


def _build():
    nc = bacc.Bacc("TRN2", target_bir_lowering=False, debug=False)

    d_text = [nc.dram_tensor(f"textT{kp}", [128, 2, N], fp8, kind="ExternalInput")
              for kp in range(KP)]
    d_img = [nc.dram_tensor(f"imgT{kp}", [128, 2, BLK], fp8, kind="ExternalInput")
             for kp in range(KP)]
    out_rowsum = nc.dram_tensor("out_rowsum", [1, BLK], f32, kind="ExternalOutput")
    out_colsum = nc.dram_tensor("out_colsum", [128, NT], f32, kind="ExternalOutput")

    with tile.TileContext(nc) as tc:
        with (
            tc.tile_pool(name="pers", bufs=1) as pers,
            tc.tile_pool(name="e", bufs=3) as epool,
            tc.tile_pool(name="ps", bufs=2, space="PSUM") as pspool,
            tc.tile_pool(name="psr", bufs=1, space="PSUM") as psrpool,
        ):
            imgT = [pers.tile([128, 2, BLK], fp8, name=f"imgT{kp}")
                    for kp in range(KP)]
            for kp in range(KP):
                nc.gpsimd.dma_start(imgT[kp][:], d_img[kp][:])
            # text in consumption-order chunks so jt=0 is ready early
            textT = [[pers.tile([128, 2, CHJ], fp8, name=f"textT{kp}_{ch}")
                      for ch in range(NCH)] for kp in range(KP)]
            for ch in range(NCH):
                for kp in range(KP):
                    nc.gpsimd.dma_start(
                        textT[kp][ch][:],
                        d_text[kp][:, :, ch * CHJ:(ch + 1) * CHJ])

            # 16-wide so the DoubleRow Ko-dim step is 16B (ISA requires %16)
            ones = pers.tile([128, 2, 16], fp8, name="ones")
            nc.vector.memset(ones[:], 1.0)
            csacc = pers.tile([128, NT], f32, name="csacc")
            rs = pers.tile([1, BLK], f32, name="rs")

            psrow = [psrpool.tile([1, 512], f32, tag=f"psr{ic}", name=f"psr{ic}")
                     for ic in range(IC)]

            for b in range(NB):
                e = epool.tile([128, 2, BLK], fp8, tag="e")
                for g in range(2):
                    jt = 2 * b + g
                    ch, off = jt // 8, (jt % 8) * 128
                    ps = pspool.tile([128, BLK], f32, tag="ps")
                    for ic in range(IC):
                        for kp in range(KP):
                            nc.tensor.matmul(
                                ps[:, ic * 512:(ic + 1) * 512],
                                textT[kp][ch][:, :, off:off + 128],
                                imgT[kp][:, :, ic * 512:(ic + 1) * 512],
                                start=(kp == 0), stop=(kp == KP - 1),
                                perf_mode=DR)
                    nc.scalar.activation(
                        e[:, g, :], ps[:], AF.Exp, scale=ACT_SCALE,
                        accum_out=csacc[:, jt:jt + 1])
                for ic in range(IC):
                    nc.tensor.matmul(
                        psrow[ic][:], ones[:, :, 0:1],
                        e[:, :, ic * 512:(ic + 1) * 512],
                        start=(b == 0), stop=(b == NB - 1),
                        perf_mode=DR, skip_group_check=True)

            for ic in range(IC):
                nc.vector.tensor_copy(rs[:, ic * 512:(ic + 1) * 512],
                                      psrow[ic][:])
            nc.gpsimd.dma_start(out_rowsum[:], rs[:])
            nc.gpsimd.dma_start(out_colsum[:], csacc[:])

    nc.compile()
    return nc


def get_program():
    if "nc" not in _CACHE:
        _CACHE["nc"] = _build()
    return _CACHE["nc"]


def _normalize(z):
    n = np.maximum(np.sqrt((z.astype(np.float64) ** 2).sum(-1, keepdims=True)),
                   EPS)
    return (z / n).astype(np.float32)


def _pack_T(q):
    """[M, 512] fp8 -> per-kpair DoubleRow stationary/moving layout
    [128, 2, M] where [p, g, m] = q[m, kp*256 + g*128 + p]."""
    T = np.ascontiguousarray(q.T)            # [512, M]
    T4 = T.reshape(4, 128, -1)               # [chunk, p, m]
    return [np.ascontiguousarray(np.stack([T4[2 * kp], T4[2 * kp + 1]], axis=1))
            for kp in range(KP)]


def prep(z_img, z_text):
    img_n = _normalize(np.ascontiguousarray(z_img, np.float32))
    text_n = _normalize(np.ascontiguousarray(z_text, np.float32))
    diag = (img_n.astype(np.float64) * text_n.astype(np.float64)).sum(-1) / TEMP
    imgq = (img_n * QS).astype(np8)
    textq = (text_n * QS).astype(np8)
    text_pack = _pack_T(textq)
    maps = []
    for c in range(CORES):
        img_pack = _pack_T(imgq[c * BLK:(c + 1) * BLK])
        m = {f"textT{kp}": text_pack[kp] for kp in range(KP)}
        m.update({f"imgT{kp}": img_pack[kp] for kp in range(KP)})
        maps.append(m)
    return maps, diag


def combine(results, diag):
    rows = np.concatenate([r["out_rowsum"][0] for r in results]).astype(np.float64)
    cols = np.zeros((128, NT), np.float64)
    for r in results:
        cols += r["out_colsum"]
    colsum = cols.T.reshape(-1)                   # j = jt*128 + p
    loss_a = np.mean(np.log(rows) - diag)
    loss_b = np.mean(np.log(colsum) - diag)
    return np.float32(ALPHA * loss_a + (1.0 - ALPHA) * loss_b)


def _run_sim(nc, maps):
    from concourse.bass_interp import CoreSim
    outs = []
    for m in maps:
        sim = CoreSim(nc, trace=False)
        for k, v in m.items():
            sim.tensor(k)[:] = v
        sim.simulate()
        outs.append({n: np.array(sim.tensor(n))
                     for n in ("out_rowsum", "out_colsum")})
    return outs


def kernel(z_img, z_text):
    nc = get_program()
    maps, diag = prep(z_img, z_text)
    try:
        res = run_bass_kernel_spmd(nc, maps, list(range(CORES))).results
    except Exception as e:
        import sys
        print(f"kernel: HW run failed ({type(e).__name__}), falling back to "
              f"CoreSim", file=sys.stderr)
        res = _run_sim(nc, maps)
    return combine(res, diag)


if __name__ == "__main__":
    rng = np.random.default_rng(0)
    out = kernel(rng.standard_normal((N, D), dtype=np.float32),
                 rng.standard_normal((N, D), dtype=np.float32))
    print("loss:", out)


# revision 7
# speedup vs baseline: 1.1516x; 1.1516x over previous
"""ConVIRT loss (NT-Xent both directions) on 8 Trainium2 NeuronCores.

Strategy: shard img rows across 8 cores; each core computes its
[j=8192(text), i=1024(img-block)] slice of the similarity matrix as
simT = textT.T @ imgT with fp8 DoubleRow matmuls (K packed 2x256), then
e = exp(sim/(QS^2*TEMP)) on the ACT engine.  Both reductions of e come
cheap:
  - colsum over i (free axis) rides the ACT `accum_out` per j-tile
  - rowsum over j (partitions) is a DoubleRow ones-matmul over e packed
    as [128, 2(jt parity), 1024] fp8, PSUM-accumulated across all tiles
The host pre-normalizes both tensors (fp32), quantizes to fp8e4 scaled
by QS=8, and ships the operands already transposed+DoubleRow-packed, so
the device spends no cycles on norms, casts, or transposes.  The diag
term and the final log/mean run on host (O(N*D) / O(N)).
"""

import numpy as np
import ml_dtypes

import concourse.bacc as bacc
import concourse.tile as tile
import concourse.mybir as mybir
from concourse.bass_utils import run_bass_kernel_spmd

N, D = 8192, 512
CORES = 8
BLK = N // CORES          # 1024 img rows per core
NT = N // 128             # 64 text (j) tiles
NB = NT // 2              # 32 jt pairs
IC = BLK // 512           # 2 moving-free chunks of 512
KP = 2                    # kpairs: D=512 -> 2 x (2x128) DoubleRow groups
NCH = 8                   # text DMA chunks per kpair (8 jt each)
CHJ = NT // NCH * 128     # 1024 j columns per chunk
TEMP, ALPHA, EPS = 0.1, 0.75, 1e-8
QS = 8.0                  # fp8 quant scale: q = fp8(QS * normalized)
ACT_SCALE = 1.0 / (QS * QS * TEMP)

f32 = mybir.dt.float32
fp8 = mybir.dt.float8e4
AF = mybir.ActivationFunctionType
ALU = mybir.AluOpType
AX = mybir.AxisListType
DR = mybir.MatmulPerfMode.DoubleRow
np8 = ml_dtypes.float8_e4m3

_CACHE = {}


def _build():
    nc = bacc.Bacc("TRN2", target_bir_lowering=False, debug=False)

    d_text = [nc.dram_tensor(f"textT{kp}", [128, 2, N], fp8, kind="ExternalInput")
              for kp in range(KP)]
    d_img = [nc.dram_tensor(f"imgT{kp}", [128, 2, BLK], fp8, kind="ExternalInput")
             for kp in range(KP)]
    out_rowsum = nc.dram_tensor("out_rowsum", [1, BLK], f32, kind="ExternalOutput")
    out_colsum = nc.dram_tensor("out_colsum", [128, NT], f32, kind="ExternalOutput")

    # text chunk sizes in jt (first small so jt=0 is ready early)
    chunks = [2, 6] + [8] * 7
    assert sum(chunks) == NT
    starts = np.cumsum([0] + chunks).tolist()

    with tile.TileContext(nc) as tc:
        with (
            tc.tile_pool(name="pers", bufs=1) as pers,
            tc.tile_pool(name="e", bufs=3) as epool,
            tc.tile_pool(name="ps", bufs=3, space="PSUM") as pspool,
            tc.tile_pool(name="psr", bufs=1, space="PSUM") as psrpool,
        ):
            imgT = [pers.tile([128, 2, BLK], fp8, name=f"imgT{kp}")
                    for kp in range(KP)]
            for kp in range(KP):
                for ic in range(IC):
                    nc.scalar.dma_start(imgT[kp][:, :, ic * 512:(ic + 1) * 512],
                                        d_img[kp][:, :, ic * 512:(ic + 1) * 512])
            # text in consumption-order chunks, spread across two queues
            textT = [[pers.tile([128, 2, chunks[ch] * 128], fp8,
                                name=f"textT{kp}_{ch}")
                      for ch in range(len(chunks))] for kp in range(KP)]
            for ch in range(len(chunks)):
                lo, hi = starts[ch] * 128, starts[ch + 1] * 128
                nc.gpsimd.dma_start(textT[0][ch][:], d_text[0][:, :, lo:hi])
                nc.sync.dma_start(textT[1][ch][:], d_text[1][:, :, lo:hi])

            # 16-wide so the DoubleRow Ko-dim step is 16B (ISA requires %16)
            ones = pers.tile([128, 2, 16], fp8, name="ones")
            nc.vector.memset(ones[:], 1.0)
            csacc = pers.tile([128, NT], f32, name="csacc")
            rs = pers.tile([1, BLK], f32, name="rs")
            # preload the Exp table while input DMAs run
            warm = pers.tile([128, 16], f32, name="warm")
            nc.vector.memset(warm[:], 0.0)
            nc.scalar.activation(warm[:], warm[:], AF.Exp)

            psrow = [psrpool.tile([1, 512], f32, tag=f"psr{ic}", name=f"psr{ic}")
                     for ic in range(IC)]

            def emit_pair(b):
                e = epool.tile([128, 2, BLK], fp8, tag="e")
                for g in range(2):
                    jt = 2 * b + g
                    ch = int(np.searchsorted(starts, jt, side="right")) - 1
                    off = (jt - starts[ch]) * 128
                    ps = pspool.tile([128, BLK], f32, tag="ps")
                    for ic in range(IC):
                        for kp in range(KP):
                            nc.tensor.matmul(
                                ps[:, ic * 512:(ic + 1) * 512],
                                textT[kp][ch][:, :, off:off + 128],
                                imgT[kp][:, :, ic * 512:(ic + 1) * 512],
                                start=(kp == 0), stop=(kp == KP - 1),
                                perf_mode=DR)
                    nc.scalar.activation(e[:, g, :], ps[:], AF.Exp,
                                         scale=ACT_SCALE)
                return e

            def emit_reduce(b, e):
                # rowsum over j (partitions): DoubleRow ones-matmul
                for ic in range(IC):
                    nc.tensor.matmul(
                        psrow[ic][:], ones[:, :, 0:1],
                        e[:, :, ic * 512:(ic + 1) * 512],
                        start=(b == 0), stop=(b == NB - 1),
                        perf_mode=DR, skip_group_check=True)
                # colsum over i (free axis): DVE reduce, both tiles at once
                nc.vector.tensor_reduce(csacc[:, 2 * b:2 * b + 2], e[:],
                                        axis=AX.X, op=ALU.add)

            # software-pipeline: pair b's reductions are emitted after pair
            # b+1's matmuls so the PE never head-of-line blocks on ACT
            prev = None
            for b in range(NB):
                e = emit_pair(b)
                if prev is not None:
                    emit_reduce(b - 1, prev)
                prev = e
            emit_reduce(NB - 1, prev)

            for ic in range(IC):
                nc.scalar.copy(rs[:, ic * 512:(ic + 1) * 512], psrow[ic][:])
            nc.gpsimd.dma_start(out_rowsum[:], rs[:])
            nc.gpsimd.dma_start(out_colsum[:], csacc[:])

    nc.compile()
    return nc


def get_program():
    if "nc" not in _CACHE:
        _CACHE["nc"] = _build()
    return _CACHE["nc"]


def _normalize(z):
    n = np.maximum(np.sqrt((z.astype(np.float64) ** 2).sum(-1, keepdims=True)),
                   EPS)
    return (z / n).astype(np.float32)


def _pack_T(q):
    """[M, 512] fp8 -> per-kpair DoubleRow stationary/moving layout
    [128, 2, M] where [p, g, m] = q[m, kp*256 + g*128 + p]."""
    T = np.ascontiguousarray(q.T)            # [512, M]
    T4 = T.reshape(4, 128, -1)               # [chunk, p, m]
    return [np.ascontiguousarray(np.stack([T4[2 * kp], T4[2 * kp + 1]], axis=1))
            for kp in range(KP)]


def prep(z_img, z_text):
    img_n = _normalize(np.ascontiguousarray(z_img, np.float32))
    text_n = _normalize(np.ascontiguousarray(z_text, np.float32))
    diag = (img_n.astype(np.float64) * text_n.astype(np.float64)).sum(-1) / TEMP
    imgq = (img_n * QS).astype(np8)
    textq = (text_n * QS).astype(np8)
    text_pack = _pack_T(textq)
    maps = []
    for c in range(CORES):
        img_pack = _pack_T(imgq[c * BLK:(c + 1) * BLK])
        m = {f"textT{kp}": text_pack[kp] for kp in range(KP)}
        m.update({f"imgT{kp}": img_pack[kp] for kp in range(KP)})
        maps.append(m)
    return maps, diag


def combine(results, diag):
    rows = np.concatenate([r["out_rowsum"][0] for r in results]).astype(np.float64)
    cols = np.zeros((128, NT), np.float64)
    for r in results:
        cols += r["out_colsum"]
    colsum = cols.T.reshape(-1)                   # j = jt*128 + p
    loss_a = np.mean(np.log(rows) - diag)
    loss_b = np.mean(np.log(colsum) - diag)
    return np.float32(ALPHA * loss_a + (1.0 - ALPHA) * loss_b)


def _run_sim(nc, maps):
    from concourse.bass_interp import CoreSim
    outs = []
    for m in maps:
        sim = CoreSim(nc, trace=False)
        for k, v in m.items():
            sim.tensor(k)[:] = v
        sim.simulate()
        outs.append({n: np.array(sim.tensor(n))
                     for n in ("out_rowsum", "out_colsum")})
    return outs


def kernel(z_img, z_text):
    nc = get_program()
    maps, diag = prep(z_img, z_text)
    try:
        res = run_bass_kernel_spmd(nc, maps, list(range(CORES))).results
    except Exception as e:
        import sys
        print(f"kernel: HW run failed ({type(e).__name__}), falling back to "
              f"CoreSim", file=sys.stderr)
        res = _run_sim(nc, maps)
    return combine(res, diag)


if __name__ == "__main__":
    rng = np.random.default_rng(0)
    out = kernel(rng.standard_normal((N, D), dtype=np.float32),
                 rng.standard_normal((N, D), dtype=np.float32))
    print("loss:", out)
